# revision 3
# baseline (speedup 1.0000x reference)
"""Per-sample dynamic 3x3 convolution (B=16, C=128, 64x64, pad 1) on 8
Trainium2 NeuronCores.

Sharding: pure data parallel — batch 16 -> 2 samples per core, no
cross-core communication.

Device kernel (per core, per sample), implicit GEMM:
  - features are host zero-padded to (128ci, 66*66) so every DMA is
    contiguous; the dynamic kernel is host pre-transposed to
    (128ci, tap*co) so each tap's (ci, co) slice is a matmul lhsT.
  - output rows are produced 8 at a time (512 px = one PSUM bank);
    each chunk accumulates 9 float32r matmuls (one per 3x3 tap) with
    shifted windows into the padded image.  float32r streams the
    moving operand at 1 col/cycle (vs 4 for plain fp32) with near-fp32
    accuracy (measured rel l2 err ~1.5e-4 end to end).
  - PSUM is evacuated by VectorE copies; HWDGE DMAs are split across
    the sync (feature loads) and scalar (weights/stores) queues; the
    feature image is loaded in 8 row-slices so chunk-0 matmuls start
    after ~1/8 of the image has landed.
"""

from contextlib import ExitStack

import numpy as np

B = 16
N_CORES = 8
BPC = B // N_CORES  # samples per core
CI = 128
CO = 128
H = W = 64
KS = 3
PADW = W + 2
PADH = H + 2
NPIX = H * W
ROWS_PER_CHUNK = 8
NCHUNK = H // ROWS_PER_CHUNK
NFREE = ROWS_PER_CHUNK * W  # 512 = one PSUM bank of fp32

_CACHE = {}


def _build_conv():
    import concourse.tile as tile
    from concourse import bacc, mybir

    F32 = mybir.dt.float32
    F32R = mybir.dt.float32r

    nc = bacc.Bacc("TRN2", target_bir_lowering=False, debug=False,
                   num_devices=N_CORES)
    feats = nc.dram_tensor("features", [BPC, CI, PADH * PADW], F32R,
                           kind="ExternalInput").ap()
    wts = nc.dram_tensor("weights", [BPC, CI, KS * KS * CO], F32R,
                         kind="ExternalInput").ap()
    out = nc.dram_tensor("out", [BPC, CO, NPIX], F32,
                         kind="ExternalOutput").ap()

    FEAT_SPLIT = 8
    with tile.TileContext(nc) as tc:
        with ExitStack() as ctx:
            xpool = ctx.enter_context(tc.tile_pool(name="xpad", bufs=2))
            wpool = ctx.enter_context(tc.tile_pool(name="wts", bufs=2))
            opool = ctx.enter_context(tc.tile_pool(name="outb", bufs=4))
            pspool = ctx.enter_context(
                tc.tile_pool(name="psum", bufs=8, space="PSUM"))

            # Issue every input DMA before any output store enters the
            # HWDGE rings: rings are FIFO per issuing engine, so an
            # out-DMA waiting on its PSUM-evacuation copy would head-of-
            # line-block sample 1's weight/feature loads (~3.4us, A/B
            # measured).
            wt_tiles, xp_views = {}, {}
            splits = np.linspace(0, PADH, FEAT_SPLIT + 1).astype(int)
            for b in range(BPC):
                wt = wpool.tile([CI, KS * KS * CO], F32R, tag="wt",
                                name=f"wt{b}")
                nc.scalar.dma_start(wt[:], wts[b])
                wt_tiles[b] = wt
            for b in range(BPC):
                xp = xpool.tile([CI, PADH * PADW], F32R, tag="xp",
                                name=f"xp{b}")
                xpv = xp[:].rearrange("p (h w) -> p h w", w=PADW)
                fv = feats[b].rearrange("p (h w) -> p h w", w=PADW)
                for s0, s1 in zip(splits[:-1], splits[1:]):
                    nc.sync.dma_start(xpv[:, s0:s1, :], fv[:, s0:s1, :])
                xp_views[b] = xpv

            for b in range(BPC):
                wt = wt_tiles[b]
                xpv = xp_views[b]
                for k in range(NCHUNK):
                    ps = pspool.tile([CO, NFREE], F32, tag="ps",
                                     name=f"ps{b}_{k}")
                    for t in range(KS * KS):
                        kh, kw = divmod(t, KS)
                        r0 = ROWS_PER_CHUNK * k + kh
                        rhs = xpv[:, r0:r0 + ROWS_PER_CHUNK, kw:kw + W]
                        nc.tensor.matmul(ps[:], wt[:, t * CO:(t + 1) * CO],
                                         rhs, start=(t == 0),
                                         stop=(t == KS * KS - 1))
                    ob = opool.tile([CO, NFREE], F32)
                    nc.vector.tensor_copy(ob[:], ps[:])
                    nc.scalar.dma_start(out[b][:, NFREE * k:NFREE * (k + 1)],
                                        ob[:])
    nc.compile()
    return nc


def _host_pack_weights(dynamic_kernel):
    w = np.ascontiguousarray(
        dynamic_kernel.astype(np.float32).transpose(0, 2, 3, 4, 1))
    return w.reshape(B, CI, KS * KS * CO)


def _host_pad_features(features):
    xp = np.zeros((B, CI, PADH, PADW), np.float32)
    xp[:, :, 1:H + 1, 1:W + 1] = features.astype(np.float32)
    return xp.reshape(B, CI, PADH * PADW)


def kernel(features, dynamic_kernel):
    """features (16,128,64,64) f32, dynamic_kernel (16,128,128,3,3) f32
    -> (16,128,64,64) f32."""
    from concourse.bass_utils import run_bass_kernel_spmd

    features = np.asarray(features)
    dynamic_kernel = np.asarray(dynamic_kernel)

    if "nc" not in _CACHE:
        _CACHE["nc"] = _build_conv()
    nc = _CACHE["nc"]

    f_padded = _host_pad_features(features)
    w_packed = _host_pack_weights(dynamic_kernel)
    in_maps = [{"features": f_padded[BPC * c:BPC * (c + 1)],
                "weights": w_packed[BPC * c:BPC * (c + 1)]}
               for c in range(N_CORES)]

    import time as _time
    last_err = None
    for attempt in range(4):  # transient NRT/device errors: retry
        try:
            res = run_bass_kernel_spmd(nc, in_maps,
                                       core_ids=list(range(N_CORES)))
            break
        except Exception as e:  # noqa: BLE001
            last_err = e
            # give the terminal time to recover a wedged core before
            # the next attempt (immediate retries hit the same state)
            _time.sleep(5 * (attempt + 1))
    else:
        raise last_err

    got = np.concatenate([res.results[c]["out"] for c in range(N_CORES)],
                         axis=0)
    return got.reshape(B, CO, H, W).astype(np.float32)


# revision 4
# speedup vs baseline: 2.7984x; 2.7984x over previous
"""Per-sample dynamic 3x3 convolution (B=16, C=128, 64x64, pad 1) on 8
Trainium2 NeuronCores.

Sharding: pure data parallel — batch 16 -> 2 samples per core, no
cross-core communication.

Device kernel (per core, per sample), implicit GEMM:
  - features are host zero-padded to (128ci, 66*66) so every DMA is
    contiguous; the dynamic kernel is host pre-transposed to
    (128ci, tap*co) so each tap's (ci, co) slice is a matmul lhsT.
  - output rows are produced 8 at a time (512 px = one PSUM bank);
    each chunk accumulates 9 float32r matmuls (one per 3x3 tap) with
    shifted windows into the padded image.  float32r streams the
    moving operand at 1 col/cycle (vs 4 for plain fp32) with near-fp32
    accuracy (measured rel l2 err ~1.5e-4 end to end).
  - PSUM is evacuated by VectorE copies; HWDGE DMAs are split across
    the sync (feature loads) and scalar (weights/stores) queues; the
    feature image is loaded in 8 row-slices so chunk-0 matmuls start
    after ~1/8 of the image has landed.
"""

from contextlib import ExitStack

import numpy as np

B = 16
N_CORES = 8
BPC = B // N_CORES  # samples per core
CI = 128
CO = 128
H = W = 64
KS = 3
PADW = W + 2
PADH = H + 2
NPIX = H * W
ROWS_PER_CHUNK = 8
NCHUNK = H // ROWS_PER_CHUNK
NFREE = ROWS_PER_CHUNK * W  # 512 = one PSUM bank of fp32

_CACHE = {}


def _build_conv():
    import concourse.tile as tile
    from concourse import bacc, mybir

    F32 = mybir.dt.float32
    F32R = mybir.dt.float32r

    nc = bacc.Bacc("TRN2", target_bir_lowering=False, debug=False,
                   num_devices=N_CORES)
    feats = nc.dram_tensor("features", [BPC, CI, PADH * PADW], F32R,
                           kind="ExternalInput").ap()
    wts = nc.dram_tensor("weights", [BPC, CI, KS * KS * CO], F32R,
                         kind="ExternalInput").ap()
    out = nc.dram_tensor("out", [BPC, CO, NPIX], F32,
                         kind="ExternalOutput").ap()

    FEAT_SPLIT = 8
    with tile.TileContext(nc) as tc:
        with ExitStack() as ctx:
            xpool = ctx.enter_context(tc.tile_pool(name="xpad", bufs=2))
            wpool = ctx.enter_context(tc.tile_pool(name="wts", bufs=2))
            opool = ctx.enter_context(tc.tile_pool(name="outb", bufs=4))
            pspool = ctx.enter_context(
                tc.tile_pool(name="psum", bufs=8, space="PSUM"))

            # Issue every input DMA before any output store enters the
            # HWDGE rings: rings are FIFO per issuing engine, so an
            # out-DMA waiting on its PSUM-evacuation copy would head-of-
            # line-block sample 1's weight/feature loads (~3.4us, A/B
            # measured).
            wt_tiles, xp_views = {}, {}
            splits = np.linspace(0, PADH, FEAT_SPLIT + 1).astype(int)
            for b in range(BPC):
                wt = wpool.tile([CI, KS * KS * CO], F32R, tag="wt",
                                name=f"wt{b}")
                if b == 0:
                    # tap-0 slice first: the very first matmul needs only
                    # wt[:, :CO], which lands ~1.4us earlier than the
                    # full 576KB weight DMA
                    nc.scalar.dma_start(wt[:, :CO], wts[b][:, :CO])
                    nc.scalar.dma_start(wt[:, CO:], wts[b][:, CO:])
                else:
                    nc.scalar.dma_start(wt[:], wts[b])
                wt_tiles[b] = wt
            for b in range(BPC):
                xp = xpool.tile([CI, PADH * PADW], F32R, tag="xp",
                                name=f"xp{b}")
                xpv = xp[:].rearrange("p (h w) -> p h w", w=PADW)
                fv = feats[b].rearrange("p (h w) -> p h w", w=PADW)
                if b == 0:
                    # 10-row first slice: covers chunk 0's padded rows
                    # 0..9 so its matmuls unblock after ~1/7 of the image
                    bounds = [0, 10] + list(
                        np.linspace(10, PADH, FEAT_SPLIT).astype(int))[1:]
                else:
                    bounds = list(splits)
                for s0, s1 in zip(bounds[:-1], bounds[1:]):
                    nc.sync.dma_start(xpv[:, s0:s1, :], fv[:, s0:s1, :])
                xp_views[b] = xpv

            for b in range(BPC):
                wt = wt_tiles[b]
                xpv = xp_views[b]
                for k in range(NCHUNK):
                    ps = pspool.tile([CO, NFREE], F32, tag="ps",
                                     name=f"ps{b}_{k}")
                    for t in range(KS * KS):
                        kh, kw = divmod(t, KS)
                        r0 = ROWS_PER_CHUNK * k + kh
                        rhs = xpv[:, r0:r0 + ROWS_PER_CHUNK, kw:kw + W]
                        nc.tensor.matmul(ps[:], wt[:, t * CO:(t + 1) * CO],
                                         rhs, start=(t == 0),
                                         stop=(t == KS * KS - 1))
                    ob = opool.tile([CO, NFREE], F32)
                    nc.vector.tensor_copy(ob[:], ps[:])
                    nc.scalar.dma_start(out[b][:, NFREE * k:NFREE * (k + 1)],
                                        ob[:])
    nc.compile()
    return nc


def _host_pack_weights(dynamic_kernel):
    w = np.ascontiguousarray(
        dynamic_kernel.astype(np.float32).transpose(0, 2, 3, 4, 1))
    return w.reshape(B, CI, KS * KS * CO)


def _host_pad_features(features):
    xp = np.zeros((B, CI, PADH, PADW), np.float32)
    xp[:, :, 1:H + 1, 1:W + 1] = features.astype(np.float32)
    return xp.reshape(B, CI, PADH * PADW)


def kernel(features, dynamic_kernel):
    """features (16,128,64,64) f32, dynamic_kernel (16,128,128,3,3) f32
    -> (16,128,64,64) f32."""
    from concourse.bass_utils import run_bass_kernel_spmd

    features = np.asarray(features)
    dynamic_kernel = np.asarray(dynamic_kernel)

    if "nc" not in _CACHE:
        _CACHE["nc"] = _build_conv()
    nc = _CACHE["nc"]

    f_padded = _host_pad_features(features)
    w_packed = _host_pack_weights(dynamic_kernel)
    in_maps = [{"features": f_padded[BPC * c:BPC * (c + 1)],
                "weights": w_packed[BPC * c:BPC * (c + 1)]}
               for c in range(N_CORES)]

    import time as _time
    last_err = None
    for attempt in range(4):  # transient NRT/device errors: retry
        try:
            res = run_bass_kernel_spmd(nc, in_maps,
                                       core_ids=list(range(N_CORES)))
            break
        except Exception as e:  # noqa: BLE001
            last_err = e
            # give the terminal time to recover a wedged core before
            # the next attempt (immediate retries hit the same state)
            _time.sleep(5 * (attempt + 1))
    else:
        raise last_err

    got = np.concatenate([res.results[c]["out"] for c in range(N_CORES)],
                         axis=0)
    return got.reshape(B, CO, H, W).astype(np.float32)
